# revision 2
# baseline (speedup 1.0000x reference)
import sys

import numpy as np

sys.path.insert(0, "/opt/trn_rl_repo")

import concourse.bass as bass  # noqa: F401
import concourse.mybir as mybir
import concourse.tile as tile
from concourse import bacc
from concourse.bass_utils import run_bass_kernel_spmd

D = H = W = 128
SIGMA = 3
K = 7
N_CORES = 8

_NC_CACHE = {}


def _blur_matrix(g: np.ndarray) -> np.ndarray:
    # Dense 128x128 operator for a clamped (edge-padded) 1D blur along a
    # length-128 axis: A[i, j] = sum of g[k] over taps where clamp(i+k-3)==j.
    A = np.zeros((D, D), dtype=np.float64)
    for i in range(D):
        for k in range(K):
            j = min(max(i + k - SIGMA, 0), D - 1)
            A[i, j] += float(g[k])
    return A.astype(np.float32)


def _build(gl):
    nc = bacc.Bacc("TRN2", target_bir_lowering=False, debug=True)
    x = nc.dram_tensor("x", [D, H, W], mybir.dt.float32, kind="ExternalInput")
    at = nc.dram_tensor("at", [D, D], mybir.dt.float32, kind="ExternalInput")
    out = nc.dram_tensor("out", [D, H, W], mybir.dt.float32, kind="ExternalOutput")

    mult = mybir.AluOpType.mult
    add = mybir.AluOpType.add

    # Clamped-edge weights: col/row 0..2 and 125..127 absorb out-of-range taps.
    lo_w = [float(sum(gl[k] for k in range(0, 3 - i))) for i in range(3)]
    hi_w = [float(sum(gl[k] for k in range(4 + m, 7))) for m in range(3)]

    with tile.TileContext(nc) as tc:
        with tc.tile_pool(name="big", bufs=1) as big, \
             tc.tile_pool(name="cst", bufs=1) as cst, \
             tc.tile_pool(name="ps", bufs=4, space="PSUM") as ps:
            att = cst.tile([D, D], mybir.dt.float32)
            nc.sync.dma_start(att[:], at[:])
            xt = big.tile([D, H * W], mybir.dt.float32)
            yt = big.tile([D, H * W], mybir.dt.float32)
            nc.sync.dma_start(xt[:], x[:].rearrange("d h w -> d (h w)"))
            x3 = xt[:].rearrange("d (h w) -> d h w", h=H)
            y3 = yt[:].rearrange("d (h w) -> d h w", h=H)

            def macc(dst, src, w):
                nc.vector.scalar_tensor_tensor(dst, src, float(w), dst, op0=mult, op1=add)

            # ---- W pass: xt -> yt (blur along last axis, clamped) ----
            nc.vector.memset(yt[:], 0.0)
            for k in range(K):
                o = k - SIGMA
                lo, hi = max(0, -o), min(W, W - o)
                macc(y3[:, :, lo:hi], x3[:, :, lo + o:hi + o], gl[k])
            for i in range(3):
                macc(y3[:, :, i:i + 1], x3[:, :, 0:1], lo_w[i])
                macc(y3[:, :, W - 1 - i:W - i], x3[:, :, W - 1:W], hi_w[i])

            # ---- D pass: yt -> xt via TensorE (A @ Y over partition axis) ----
            CF = 512  # 512 f32 -> one PSUM bank
            for c in range(H * W // CF):
                pt = ps.tile([D, CF], mybir.dt.float32)
                nc.tensor.matmul(pt[:], att[:], yt[:, c * CF:(c + 1) * CF],
                                 start=True, stop=True)
                nc.any.tensor_copy(xt[:, c * CF:(c + 1) * CF], pt[:])

            # ---- H pass: xt -> yt (blur along middle axis, clamped) ----
            nc.vector.memset(yt[:], 0.0)
            for k in range(K):
                o = k - SIGMA
                lo, hi = max(0, -o), min(H, H - o)
                macc(y3[:, lo:hi, :], x3[:, lo + o:hi + o, :], gl[k])
            for i in range(3):
                macc(y3[:, i:i + 1, :], x3[:, 0:1, :], lo_w[i])
                macc(y3[:, H - 1 - i:H - i, :], x3[:, H - 1:H, :], hi_w[i])

            nc.sync.dma_start(out[:].rearrange("d h w -> d (h w)"), yt[:])
    nc.finalize()
    return nc


def kernel(x, g, sigma):
    x = np.ascontiguousarray(np.asarray(x, dtype=np.float32))
    g = np.asarray(g, dtype=np.float64)
    gl = [float(v) for v in g]
    key = tuple(gl)
    if key not in _NC_CACHE:
        _NC_CACHE[key] = _build(gl)
    nc = _NC_CACHE[key]
    AT = np.ascontiguousarray(_blur_matrix(g).T)
    slabs = x.reshape(N_CORES, D, H, W)
    in_maps = [{"x": np.ascontiguousarray(slabs[i]), "at": AT} for i in range(N_CORES)]
    res = run_bass_kernel_spmd(nc, in_maps, core_ids=list(range(N_CORES)))
    global LAST_RESULT
    LAST_RESULT = res
    outs = np.stack([res.results[i]["out"] for i in range(N_CORES)])
    return outs.reshape(2, 4, D, H, W).astype(np.float32)


LAST_RESULT = None



# revision 6
# speedup vs baseline: 2.9898x; 2.9898x over previous
import sys

import numpy as np

sys.path.insert(0, "/opt/trn_rl_repo")

import concourse.bass as bass  # noqa: F401
import concourse.mybir as mybir
import concourse.tile as tile
from concourse import bacc
from concourse.bass_utils import run_bass_kernel_spmd

D = H = W = 128
SIGMA = 3
K = 7
N_CORES = 8

HC = 16  # h rows per inbound DMA chunk (1 MiB f32)
GC = 16  # d' columns per outbound group (1 MiB f32)

_NC_CACHE = {}


def _blur_matrix(g: np.ndarray) -> np.ndarray:
    # Dense 128x128 operator for a clamped (edge-padded) 1D blur along a
    # length-128 axis: A[i, j] = sum of g[k] over taps where clamp(i+k-3)==j.
    A = np.zeros((D, D), dtype=np.float64)
    for i in range(D):
        for k in range(K):
            j = min(max(i + k - SIGMA, 0), D - 1)
            A[i, j] += float(g[k])
    return A


def _build():
    nc = bacc.Bacc("TRN2", target_bir_lowering=False, debug=True)
    x = nc.dram_tensor("x", [D, H, W], mybir.dt.float32, kind="ExternalInput")
    at = nc.dram_tensor("at", [D, D], mybir.dt.float16, kind="ExternalInput")
    out = nc.dram_tensor("out", [D, H, W], mybir.dt.float32, kind="ExternalOutput")

    f16 = mybir.dt.float16
    f32 = mybir.dt.float32

    with tile.TileContext(nc) as tc:
        with tc.tile_pool(name="big", bufs=1) as big, \
             tc.tile_pool(name="cst", bufs=1) as cst, \
             tc.tile_pool(name="sin", bufs=2) as sin, \
             tc.tile_pool(name="sout", bufs=3) as sout, \
             tc.tile_pool(name="pst", bufs=2, space="PSUM") as pst, \
             tc.tile_pool(name="pss", bufs=2, space="PSUM") as pss:
            att = cst.tile([D, D], f16)
            nc.sync.dma_start(att[:], at[:])

            xh = big.tile([D, H * W], f16)   # (d, h*128 + w)
            yt = big.tile([D, H * W], f16)   # (w, d'*128 + h)
            zt = big.tile([D, H * W], f16)   # (h, d'*128 + w')
            # view of Y with free dims ordered (h, d') for the P1 copy
            y_hd = yt[:].rearrange("w (d h) -> w h d", h=H)
            out_v = out[:].rearrange("d h w -> h d w")

            ci = 0

            def evac(dst, src):
                nonlocal ci
                if ci % 2 == 0:
                    nc.vector.tensor_copy(dst, src)
                else:
                    nc.scalar.copy(dst, src)
                ci += 1

            # ---- Phase A: DMA-in + f32->f16 convert + P1 (blur D, transpose) ----
            for c in range(H // HC):
                st = sin.tile([D, HC * W], f32)
                nc.sync.dma_start(
                    st[:], x[:, c * HC:(c + 1) * HC, :].rearrange("d h w -> d (h w)"))
                nc.gpsimd.tensor_copy(xh[:, c * HC * W:(c + 1) * HC * W], st[:])
                for gi in range(HC // 8):
                    pt = pst.tile([D, 1024], f32)
                    h0 = c * HC + gi * 8
                    for j in range(8):
                        h = h0 + j
                        nc.tensor.matmul(pt[:, j * 128:(j + 1) * 128],
                                         xh[:, h * 128:(h + 1) * 128], att[:],
                                         start=True, stop=True)
                    # pt free order is (h, d'); dst enumerates (h, d') too
                    evac(y_hd[:, h0:h0 + 8, :], pt[:])

            # ---- Phase B/C: P2 (blur W, transpose) + P3 (blur H) + DMA-out ----
            for g in range(D // GC):
                for k in range(GC // 8):
                    pt = pst.tile([D, 1024], f32)
                    d0 = g * GC + k * 8
                    for j in range(8):
                        dd = d0 + j
                        nc.tensor.matmul(pt[:, j * 128:(j + 1) * 128],
                                         yt[:, dd * 128:(dd + 1) * 128], att[:],
                                         start=True, stop=True)
                    evac(zt[:, d0 * 128:d0 * 128 + 1024], pt[:])
                so = sout.tile([D, GC * 128], f32)
                for k in range(GC * 128 // 1024):
                    ps = pss.tile([D, 1024], f32)
                    n0 = g * GC * 128 + k * 1024
                    nc.tensor.matmul(ps[:, 0:512], att[:], zt[:, n0:n0 + 512],
                                     start=True, stop=True)
                    nc.tensor.matmul(ps[:, 512:1024], att[:], zt[:, n0 + 512:n0 + 1024],
                                     start=True, stop=True)
                    evac(so[:, k * 1024:(k + 1) * 1024], ps[:])
                nc.sync.dma_start(out_v[:, g * GC:(g + 1) * GC, :], so[:])
    nc.finalize()
    return nc


def kernel(x, g, sigma):
    x = np.ascontiguousarray(np.asarray(x, dtype=np.float32))
    g = np.asarray(g, dtype=np.float64)
    key = tuple(float(v) for v in g)
    if key not in _NC_CACHE:
        _NC_CACHE[key] = _build()
    nc = _NC_CACHE[key]
    AT = np.ascontiguousarray(_blur_matrix(g).T.astype(np.float16))
    slabs = x.reshape(N_CORES, D, H, W)
    in_maps = [{"x": np.ascontiguousarray(slabs[i]), "at": AT} for i in range(N_CORES)]
    res = run_bass_kernel_spmd(nc, in_maps, core_ids=list(range(N_CORES)))
    global LAST_RESULT
    LAST_RESULT = res
    outs = np.stack([res.results[i]["out"] for i in range(N_CORES)])
    return outs.reshape(2, 4, D, H, W).astype(np.float32)


LAST_RESULT = None


# revision 9
# speedup vs baseline: 4.2121x; 1.4088x over previous
import sys

import numpy as np

sys.path.insert(0, "/opt/trn_rl_repo")

import concourse.bass as bass  # noqa: F401
import concourse.mybir as mybir
import concourse.tile as tile
from concourse import bacc
from concourse.bass_utils import run_bass_kernel_spmd

D = H = W = 128
SIGMA = 3
K = 7
N_CORES = 8

HC = 16  # h rows per inbound DMA chunk (1 MiB f32)
GC = 16  # d' columns per outbound group (1 MiB f32)

_NC_CACHE = {}


def _blur_matrix(g: np.ndarray) -> np.ndarray:
    # Dense 128x128 operator for a clamped (edge-padded) 1D blur along a
    # length-128 axis: A[i, j] = sum of g[k] over taps where clamp(i+k-3)==j.
    A = np.zeros((D, D), dtype=np.float64)
    for i in range(D):
        for k in range(K):
            j = min(max(i + k - SIGMA, 0), D - 1)
            A[i, j] += float(g[k])
    return A


def _build():
    nc = bacc.Bacc("TRN2", target_bir_lowering=False, debug=True)
    x = nc.dram_tensor("x", [D, H, W], mybir.dt.float32, kind="ExternalInput")
    at = nc.dram_tensor("at", [D, D], mybir.dt.float16, kind="ExternalInput")
    out = nc.dram_tensor("out", [D, H, W], mybir.dt.float32, kind="ExternalOutput")

    f16 = mybir.dt.float16
    f32 = mybir.dt.float32

    with tile.TileContext(nc) as tc:
        with tc.tile_pool(name="big", bufs=1) as big, \
             tc.tile_pool(name="cst", bufs=1) as cst, \
             tc.tile_pool(name="sout", bufs=3) as sout, \
             tc.tile_pool(name="pst", bufs=3, space="PSUM") as pst, \
             tc.tile_pool(name="pss", bufs=2, space="PSUM") as pss:
            att = cst.tile([D, D], f16)
            nc.sync.dma_start(att[:], at[:])

            xh = big.tile([D, H * W], f16)   # (d, h*128 + w)
            yt = big.tile([D, H * W], f16)   # (w, h*128 + d')
            zt = big.tile([D, H * W], f16)   # (h, d'*128 + w')
            # view of Y as (w, h, d') for P2's strided lhsT slices
            y3 = yt[:].rearrange("w (h d) -> w h d", h=H)
            out_v = out[:].rearrange("d h w -> h d w")

            ci = 0

            def evac(dst, src):
                nonlocal ci
                if ci % 2 == 0:
                    nc.vector.tensor_copy(dst, src)
                else:
                    nc.scalar.copy(dst, src)
                ci += 1

            # ---- Phase A: DMA-in (cast f32->f16 in SWDGE) + P1 (blur D, transpose) ----
            for c in range(H // HC):
                nc.gpsimd.dma_start(
                    xh[:, c * HC * W:(c + 1) * HC * W],
                    x[:, c * HC:(c + 1) * HC, :].rearrange("d h w -> d (h w)"))
                for gi in range(HC // 8):
                    pt = pst.tile([D, 1024], f32)
                    h0 = c * HC + gi * 8
                    for j in range(8):
                        h = h0 + j
                        nc.tensor.matmul(pt[:, j * 128:(j + 1) * 128],
                                         xh[:, h * 128:(h + 1) * 128], att[:],
                                         start=True, stop=True)
                    # pt free order is (h, d'); Y free is (h*128 + d') contiguous
                    evac(yt[:, h0 * 128:h0 * 128 + 1024], pt[:])

            # ---- Phase B/C: P2 (blur W, transpose) + P3 (blur H) + DMA-out ----
            for g in range(D // GC):
                for k in range(GC // 8):
                    pt = pst.tile([D, 1024], f32)
                    d0 = g * GC + k * 8
                    for j in range(8):
                        dd = d0 + j
                        nc.tensor.matmul(pt[:, j * 128:(j + 1) * 128],
                                         y3[:, :, dd], att[:],
                                         start=True, stop=True)
                    evac(zt[:, d0 * 128:d0 * 128 + 1024], pt[:])
                so = sout.tile([D, GC * 128], f32)
                for k in range(GC * 128 // 512):
                    ps = pss.tile([D, 512], f32)
                    n0 = g * GC * 128 + k * 512
                    nc.tensor.matmul(ps[:], att[:], zt[:, n0:n0 + 512],
                                     start=True, stop=True)
                    evac(so[:, k * 512:(k + 1) * 512], ps[:])
                nc.sync.dma_start(out_v[:, g * GC:(g + 1) * GC, :], so[:])
    nc.finalize()
    return nc


def kernel(x, g, sigma):
    x = np.ascontiguousarray(np.asarray(x, dtype=np.float32))
    g = np.asarray(g, dtype=np.float64)
    key = tuple(float(v) for v in g)
    if key not in _NC_CACHE:
        _NC_CACHE[key] = _build()
    nc = _NC_CACHE[key]
    AT = np.ascontiguousarray(_blur_matrix(g).T.astype(np.float16))
    slabs = x.reshape(N_CORES, D, H, W)
    in_maps = [{"x": np.ascontiguousarray(slabs[i]), "at": AT} for i in range(N_CORES)]
    res = run_bass_kernel_spmd(nc, in_maps, core_ids=list(range(N_CORES)))
    global LAST_RESULT
    LAST_RESULT = res
    outs = np.stack([res.results[i]["out"] for i in range(N_CORES)])
    return outs.reshape(2, 4, D, H, W).astype(np.float32)


LAST_RESULT = None


# revision 12
# speedup vs baseline: 4.6257x; 1.0982x over previous
import sys

import numpy as np

sys.path.insert(0, "/opt/trn_rl_repo")

import concourse.bass as bass  # noqa: F401
import concourse.mybir as mybir
import concourse.tile as tile
from concourse import bacc
from concourse.bass_utils import run_bass_kernel_spmd

D = H = W = 128
SIGMA = 3
K = 7
N_CORES = 8

HC = 16  # h rows per inbound DMA chunk (1 MiB f32)
GC = 16  # d' columns per outbound group (1 MiB f32)

_NC_CACHE = {}


def _blur_matrix(g: np.ndarray) -> np.ndarray:
    # Dense 128x128 operator for a clamped (edge-padded) 1D blur along a
    # length-128 axis: A[i, j] = sum of g[k] over taps where clamp(i+k-3)==j.
    A = np.zeros((D, D), dtype=np.float64)
    for i in range(D):
        for k in range(K):
            j = min(max(i + k - SIGMA, 0), D - 1)
            A[i, j] += float(g[k])
    return A


def _build():
    nc = bacc.Bacc("TRN2", target_bir_lowering=False, debug=True)
    x = nc.dram_tensor("x", [D, H, W], mybir.dt.float32, kind="ExternalInput")
    at = nc.dram_tensor("at", [D, D], mybir.dt.float16, kind="ExternalInput")
    out = nc.dram_tensor("out", [D, H, W], mybir.dt.float32, kind="ExternalOutput")

    f16 = mybir.dt.float16
    f32 = mybir.dt.float32

    with tile.TileContext(nc) as tc:
        with tc.tile_pool(name="big", bufs=1) as big, \
             tc.tile_pool(name="cst", bufs=1) as cst, \
             tc.tile_pool(name="sout", bufs=3) as sout, \
             tc.tile_pool(name="pst", bufs=3, space="PSUM") as pst, \
             tc.tile_pool(name="pss", bufs=2, space="PSUM") as pss:
            att = cst.tile([D, D], f16)
            nc.sync.dma_start(att[:], at[:])

            xh = big.tile([D, H * W], f16)   # (d, h*128 + w)
            yt = big.tile([D, H * W], f16)   # (w, d'*128 + h)
            zt = big.tile([D, H * W], f16)   # (h, d'*128 + w')
            # view of Y as (w, d', h) for the P1 evacuation dst
            y3 = yt[:].rearrange("w (d h) -> w d h", h=H)
            out_v = out[:].rearrange("d h w -> h d w")

            ci = 0

            def evac(dst, src):
                nonlocal ci
                if ci % 2 == 0:
                    nc.vector.tensor_copy(dst, src)
                else:
                    nc.scalar.copy(dst, src)
                ci += 1

            # ---- Phase A: DMA-in (cast f32->f16 in SWDGE) + P1 (blur D, transpose) ----
            for c in range(H // HC):
                nc.gpsimd.dma_start(
                    xh[:, c * HC * W:(c + 1) * HC * W],
                    x[:, c * HC:(c + 1) * HC, :].rearrange("d h w -> d (h w)"))
                for gi in range(HC // 8):
                    pt = pst.tile([D, 1024], f32)
                    h0 = c * HC + gi * 8
                    for j in range(8):
                        h = h0 + j
                        nc.tensor.matmul(pt[:, j * 128:(j + 1) * 128],
                                         xh[:, h * 128:(h + 1) * 128], att[:],
                                         start=True, stop=True)
                    # enumerate (d' outer, h inner): strided f32 PSUM reads,
                    # 8x2B contiguous write runs into Y's (d'*128 + h) layout
                    evac(y3[:, :, h0:h0 + 8],
                         pt[:].rearrange("w (h d) -> w d h", h=8))

            # ---- Phase B/C: P2 (blur W, transpose) + P3 (blur H) + DMA-out ----
            for g in range(D // GC):
                for k in range(GC // 8):
                    pt = pst.tile([D, 1024], f32)
                    d0 = g * GC + k * 8
                    for j in range(8):
                        dd = d0 + j
                        nc.tensor.matmul(pt[:, j * 128:(j + 1) * 128],
                                         yt[:, dd * 128:(dd + 1) * 128], att[:],
                                         start=True, stop=True)
                    evac(zt[:, d0 * 128:d0 * 128 + 1024], pt[:])
                so = sout.tile([D, GC * 128], f32)
                for k in range(GC * 128 // 512):
                    ps = pss.tile([D, 512], f32)
                    n0 = g * GC * 128 + k * 512
                    nc.tensor.matmul(ps[:], att[:], zt[:, n0:n0 + 512],
                                     start=True, stop=True)
                    evac(so[:, k * 512:(k + 1) * 512], ps[:])
                nc.sync.dma_start(out_v[:, g * GC:(g + 1) * GC, :], so[:])
    nc.finalize()
    return nc


def kernel(x, g, sigma):
    x = np.ascontiguousarray(np.asarray(x, dtype=np.float32))
    g = np.asarray(g, dtype=np.float64)
    key = tuple(float(v) for v in g)
    if key not in _NC_CACHE:
        _NC_CACHE[key] = _build()
    nc = _NC_CACHE[key]
    AT = np.ascontiguousarray(_blur_matrix(g).T.astype(np.float16))
    slabs = x.reshape(N_CORES, D, H, W)
    in_maps = [{"x": np.ascontiguousarray(slabs[i]), "at": AT} for i in range(N_CORES)]
    res = run_bass_kernel_spmd(nc, in_maps, core_ids=list(range(N_CORES)))
    global LAST_RESULT
    LAST_RESULT = res
    outs = np.stack([res.results[i]["out"] for i in range(N_CORES)])
    return outs.reshape(2, 4, D, H, W).astype(np.float32)


LAST_RESULT = None


# revision 14
# speedup vs baseline: 4.6881x; 1.0135x over previous
import sys

import numpy as np

sys.path.insert(0, "/opt/trn_rl_repo")

import concourse.bass as bass  # noqa: F401
import concourse.mybir as mybir
import concourse.tile as tile
from concourse import bacc
from concourse.bass_utils import run_bass_kernel_spmd

D = H = W = 128
SIGMA = 3
K = 7
N_CORES = 8

HC = 8   # h rows per inbound DMA chunk (0.5 MiB f32)
GC = 16  # d' columns per outbound group

_NC_CACHE = {}


def _blur_matrix(g: np.ndarray) -> np.ndarray:
    # Dense 128x128 operator for a clamped (edge-padded) 1D blur along a
    # length-128 axis: A[i, j] = sum of g[k] over taps where clamp(i+k-3)==j.
    A = np.zeros((D, D), dtype=np.float64)
    for i in range(D):
        for k in range(K):
            j = min(max(i + k - SIGMA, 0), D - 1)
            A[i, j] += float(g[k])
    return A


def _build():
    nc = bacc.Bacc("TRN2", target_bir_lowering=False, debug=True)
    x = nc.dram_tensor("x", [D, H, W], mybir.dt.float32, kind="ExternalInput")
    at = nc.dram_tensor("at", [D, D], mybir.dt.float16, kind="ExternalInput")
    out = nc.dram_tensor("out", [D, H, W], mybir.dt.float32, kind="ExternalOutput")

    f16 = mybir.dt.float16
    f32 = mybir.dt.float32

    with tile.TileContext(nc) as tc:
        with tc.tile_pool(name="big", bufs=1) as big, \
             tc.tile_pool(name="cst", bufs=1) as cst, \
             tc.tile_pool(name="sout", bufs=3) as sout, \
             tc.tile_pool(name="pst", bufs=3, space="PSUM") as pst, \
             tc.tile_pool(name="pss", bufs=2, space="PSUM") as pss:
            att = cst.tile([D, D], f16)
            nc.sync.dma_start(att[:], at[:])

            xh = big.tile([D, H * W], f16)   # (d, h*128 + w)
            yt = big.tile([D, H * W], f16)   # (w, d'*128 + h)
            zt = big.tile([D, H * W], f16)   # (h, d'*128 + w')
            # view of Y as (w, d', h) for the P1 evacuation dst
            y3 = yt[:].rearrange("w (d h) -> w d h", h=H)
            out_v = out[:].rearrange("d h w -> h d w")

            ci = 0

            def evac(dst, src):
                nonlocal ci
                if ci % 2 == 0:
                    nc.vector.tensor_copy(dst, src)
                else:
                    nc.scalar.copy(dst, src)
                ci += 1

            # ---- Phase A: DMA-in (cast f32->f16 in SWDGE) + P1 (blur D, transpose) ----
            for c in range(H // HC):
                nc.gpsimd.dma_start(
                    xh[:, c * HC * W:(c + 1) * HC * W],
                    x[:, c * HC:(c + 1) * HC, :].rearrange("d h w -> d (h w)"))
                for gi in range(HC // 8):
                    pt = pst.tile([D, 1024], f32)
                    h0 = c * HC + gi * 8
                    for j in range(8):
                        h = h0 + j
                        nc.tensor.matmul(pt[:, j * 128:(j + 1) * 128],
                                         xh[:, h * 128:(h + 1) * 128], att[:],
                                         start=True, stop=True)
                    # enumerate (d' outer, h inner): strided f32 PSUM reads,
                    # 8x2B contiguous write runs into Y's (d'*128 + h) layout
                    evac(y3[:, :, h0:h0 + 8],
                         pt[:].rearrange("w (h d) -> w d h", h=8))

            # ---- Phase B/C: P2 (blur W, transpose) + P3 (blur H) + DMA-out ----
            for g in range(D // GC):
                for k in range(GC // 8):
                    pt = pst.tile([D, 1024], f32)
                    d0 = g * GC + k * 8
                    for j in range(8):
                        dd = d0 + j
                        nc.tensor.matmul(pt[:, j * 128:(j + 1) * 128],
                                         yt[:, dd * 128:(dd + 1) * 128], att[:],
                                         start=True, stop=True)
                    evac(zt[:, d0 * 128:d0 * 128 + 1024], pt[:])
                for half in range(2):
                    so = sout.tile([D, GC * 64], f32)
                    for k in range(GC * 128 // 1024):
                        ps = pss.tile([D, 512], f32)
                        n0 = g * GC * 128 + half * GC * 64 + k * 512
                        nc.tensor.matmul(ps[:], att[:], zt[:, n0:n0 + 512],
                                         start=True, stop=True)
                        evac(so[:, k * 512:(k + 1) * 512], ps[:])
                    d0 = g * GC + half * (GC // 2)
                    nc.sync.dma_start(out_v[:, d0:d0 + GC // 2, :], so[:])
    nc.finalize()
    return nc


def kernel(x, g, sigma):
    x = np.ascontiguousarray(np.asarray(x, dtype=np.float32))
    g = np.asarray(g, dtype=np.float64)
    key = tuple(float(v) for v in g)
    if key not in _NC_CACHE:
        _NC_CACHE[key] = _build()
    nc = _NC_CACHE[key]
    AT = np.ascontiguousarray(_blur_matrix(g).T.astype(np.float16))
    slabs = x.reshape(N_CORES, D, H, W)
    in_maps = [{"x": np.ascontiguousarray(slabs[i]), "at": AT} for i in range(N_CORES)]
    res = run_bass_kernel_spmd(nc, in_maps, core_ids=list(range(N_CORES)))
    global LAST_RESULT
    LAST_RESULT = res
    outs = np.stack([res.results[i]["out"] for i in range(N_CORES)])
    return outs.reshape(2, 4, D, H, W).astype(np.float32)


LAST_RESULT = None
